# revision 8
# baseline (speedup 1.0000x reference)
"""DigitCaps (CapsNet dynamic routing) kernel for 8 Trainium2 NeuronCores.

Reference math:
  u_hat[b,r,c,o] = sum_i W[r,c,o,i] * x[b,r,i]
  b_ij = 0;  3 routing iterations:
     c = softmax_r(b);  s[b,c,o] = sum_r c[r,c] u_hat[b,r,c,o];
     v = squash(s);     b += mean_b(sum_o u_hat[b,r,c,o] v[b,c,o])
  returns v[..., None]  (256, 10, 16, 1)

Strategy: ROUTE-parallel over r (144 routes per core), batch kept whole
(B=256) on every core.  u_hat is never materialized; the routing
coefficients c are folded into the weights, so each pass is a dense
matmul over the local contraction dim K=(r_local,i)=1152 (9 tiles of
128):
    s-matmul:  s[b,(o,c)]   = sum_K XT[K,b] * (c-scaled Wg)[K,(o,c)]
               + AllReduce(s) over the 8 cores (sum over all routes)
    G-matmul:  G[K,(o,c)]   = sum_b XN[b,K] * (v[b,(o,c)]/B)
    agreement: P = Wg .* G; an indicator matmul (CD: 128->16) reduces i
               inside each 128-partition tile; a strided tensor_reduce
               over o gives abar[rr,(t,c)] = mean_b(a_ij), complete
               locally (no collective needed for abar itself).
    softmax over ALL routes: local exp/sums + AllReduce of the tiny
               (1,C) denominator; c replicated over i by one 16->128
               indicator matmul (REP), then folded into Wg.
All heavy matmuls run in bf16 (fp32 matmul is 4x slower); softmax,
squash and the s AllReduce run in fp32.  Host-side numpy prepares all
SBUF layouts.  The layout choices minimize instruction count (the
dominant cost): ~166 instructions total vs ~800 for the batch-parallel
variant.
"""
import sys
if '/opt/trn_rl_repo' not in sys.path:
    sys.path.insert(0, '/opt/trn_rl_repo')
import numpy as np
import ml_dtypes

import concourse.bass as bass
import concourse.bacc as bacc
import concourse.mybir as mybir
import concourse.tile as tile
from concourse import bass_utils

BF16 = mybir.dt.bfloat16
F32 = mybir.dt.float32

B, R, C, O, I = 256, 1152, 10, 16, 8
NCORES = 8
RL = R // NCORES          # 144 local routes
KL = RL * I               # 1152 local contraction
KT = KL // 128            # 9 K tiles
RT = 128 // I             # 16 routes per K tile
CO = C * O                # 160, free order (o,c): idx = o*C + c
NBH = B // 128            # 2 batch halves
NITER = 3
GRP = 3                   # K tiles per PSUM group (G/COMP phase)
NG = KT // GRP            # 3 groups

_CACHE = {}


def _build(n_cores=NCORES, reps=1):
    nc = bacc.Bacc("TRN2", target_bir_lowering=False, debug=False,
                   num_devices=n_cores)
    wg_d = nc.dram_tensor("wg", [KL, CO], BF16, kind="ExternalInput")
    xt_d = nc.dram_tensor("xt", [128, KT * B], BF16, kind="ExternalInput")
    xn_d = nc.dram_tensor("xn", [128, NBH * KL], BF16, kind="ExternalInput")
    rep_d = nc.dram_tensor("rep", [RT, 128], F32, kind="ExternalInput")
    cd_d = nc.dram_tensor("cd", [128, RT], BF16, kind="ExternalInput")
    out_d = nc.dram_tensor("out", [B, CO], F32, kind="ExternalOutput")

    with tile.TileContext(nc) as tc:
        with (
            tc.tile_pool(name="big", bufs=1) as big,
            tc.tile_pool(name="small", bufs=1) as small,
            tc.tile_pool(name="sps", bufs=1, space="PSUM") as sps,
            tc.tile_pool(name="gps", bufs=2, space="PSUM") as gps,
            tc.tile_pool(name="aps", bufs=1, space="PSUM") as aps,
            tc.tile_pool(name="zps", bufs=1, space="PSUM") as zps_pool,
            tc.tile_pool(name="dram", bufs=4, space="DRAM") as dram,
        ):
            Wg = big.tile([128, KT * CO], BF16, tag="Wg")
            Wp = big.tile([128, KT * CO], BF16, tag="Wp")
            Psb = big.tile([128, KT * CO], BF16, tag="Psb")
            XT = big.tile([128, KT * B], BF16, tag="XT")
            XN = big.tile([128, NBH * KL], BF16, tag="XN")
            REP = big.tile([RT, 128], F32, tag="REP")
            CDm = big.tile([128, RT], BF16, tag="CDm")

            se = big.tile([128, NBH * CO], F32, tag="se")
            t1 = big.tile([128, NBH * CO], F32, tag="t1")
            t2 = big.tile([128, NBH * CO], F32, tag="t2")
            den = big.tile([128, NBH * CO], F32, tag="den")
            num = big.tile([128, NBH * CO], F32, tag="num")
            qq = big.tile([128, NBH * CO], F32, tag="qq")
            vbf = big.tile([128, NBH * CO], BF16, tag="vbf")

            b_sb = small.tile([RT, KT * C], F32, tag="b")
            expb = small.tile([RT, KT * C], F32, tag="expb")
            abar = small.tile([RT, KT * C], F32, tag="abar")
            c_sb = small.tile([RT, KT * C], F32, tag="c")
            ones16 = small.tile([RT, 1], F32, tag="ones16")
            ones1 = small.tile([1, RT], F32, tag="ones1")
            zr = small.tile([1, C], F32, tag="zr")

            for rep in range(reps):
                nc.sync.dma_start(
                    out=Wg[:, :].rearrange("p (t f) -> p t f", f=CO),
                    in_=wg_d[:, :].rearrange("(t p) f -> p t f", p=128))
                nc.sync.dma_start(out=XT[:, :], in_=xt_d[:, :])
                nc.sync.dma_start(out=XN[:, :], in_=xn_d[:, :])
                nc.sync.dma_start(out=REP[:, :], in_=rep_d[:, :])
                nc.sync.dma_start(out=CDm[:, :], in_=cd_d[:, :])
                nc.vector.memset(ones16[:, :], 1.0)
                nc.vector.memset(ones1[:, :], 1.0)

                for k in range(NITER):
                    if k > 0:
                        # --- softmax over ALL routes: c = exp(b)/Z ---
                        nc.scalar.activation(expb[:, :], b_sb[:, :],
                                             mybir.ActivationFunctionType.Exp)
                        zp = zps_pool.tile([1, KT * C], F32, tag="zp")
                        nc.tensor.matmul(zp[:, :], ones16[:, :], expb[:, :],
                                         start=True, stop=True)
                        nc.vector.tensor_reduce(
                            zr[:, :],
                            zp[:, :].rearrange("q (t c) -> q c t", c=C),
                            axis=mybir.AxisListType.X,
                            op=mybir.AluOpType.add)
                        z_in = dram.tile([1, C], F32, tag="zin")
                        z_out = dram.tile([1, C], F32, tag="zout")
                        nc.sync.dma_start(out=z_in[:, :], in_=zr[:, :])
                        nc.gpsimd.collective_compute(
                            "AllReduce", mybir.AluOpType.add,
                            replica_groups=[list(range(n_cores))],
                            ins=[z_in[:, :].opt()],
                            outs=[z_out[:, :].opt()])
                        nc.sync.dma_start(out=zr[:, :], in_=z_out[:, :])
                        nc.vector.reciprocal(zr[:, :], zr[:, :])
                        zbc = zps_pool.tile([RT, C], F32, tag="zbc")
                        nc.tensor.matmul(zbc[:, :], ones1[:, :], zr[:, :],
                                         start=True, stop=True)
                        nc.vector.tensor_tensor(
                            c_sb[:, :].rearrange("m (t c) -> m t c", c=C),
                            expb[:, :].rearrange("m (t c) -> m t c", c=C),
                            zbc[:, :].unsqueeze(1).broadcast_to((RT, KT, C)),
                            op=mybir.AluOpType.mult)
                        # replicate c over i (16 -> 128 partitions)
                        crep = zps_pool.tile([128, KT * C], F32, tag="crep")
                        nc.tensor.matmul(crep[:, :], REP[:, :], c_sb[:, :],
                                         start=True, stop=True)
                        # W' = Wg * crep (broadcast over o)
                        nc.vector.tensor_tensor(
                            Wp[:, :].rearrange("p (t o c) -> p t o c",
                                               o=O, c=C),
                            Wg[:, :].rearrange("p (t o c) -> p t o c",
                                               o=O, c=C),
                            crep[:, :].rearrange("p (t c) -> p t c", c=C)
                            .unsqueeze(2).broadcast_to((128, KT, O, C)),
                            op=mybir.AluOpType.mult)

                    # --- s matmul: s[b,(o,c)] = sum_K XT^T (c.Wg) ---
                    mov = Wg if k == 0 else Wp
                    xt4 = XT[:, :].rearrange("p (t h b) -> p t h b",
                                             h=NBH, b=128)
                    s_ps = sps.tile([128, NBH * CO], F32, tag="s")
                    for bh in range(NBH):
                        for t in range(KT):
                            nc.tensor.matmul(
                                s_ps[:, bh * CO:(bh + 1) * CO],
                                xt4[:, t, bh, :],
                                mov[:, t * CO:(t + 1) * CO],
                                start=(t == 0), stop=(t == KT - 1))
                    # AllReduce s over all cores (sum over all routes)
                    nc.scalar.copy(se[:, :], s_ps[:, :])
                    s_in = dram.tile([B, CO], F32, tag="sin")
                    s_out = dram.tile([B, CO], F32, tag="sout")
                    nc.sync.dma_start(
                        out=s_in[:, :].rearrange("(h p) f -> p h f", p=128),
                        in_=se[:, :].rearrange("p (h f) -> p h f", f=CO))
                    nc.gpsimd.collective_compute(
                        "AllReduce", mybir.AluOpType.add,
                        replica_groups=[list(range(n_cores))],
                        ins=[s_in[:, :].opt()],
                        outs=[s_out[:, :].opt()])
                    nc.sync.dma_start(
                        out=se[:, :].rearrange("p (h f) -> p h f", f=CO),
                        in_=s_out[:, :].rearrange("(h p) f -> p h f", p=128))

                    # --- squash: v = s'|s'| / (1+s'^2), s' = alpha*s ---
                    alpha = 1.0 / R if k == 0 else 1.0
                    nc.scalar.activation(t1[:, :], se[:, :],
                                         mybir.ActivationFunctionType.Square,
                                         scale=alpha)
                    nc.scalar.activation(t2[:, :], se[:, :],
                                         mybir.ActivationFunctionType.Abs,
                                         scale=alpha)
                    nc.vector.tensor_scalar_add(den[:, :], t1[:, :], 1.0)
                    nc.vector.reciprocal(den[:, :], den[:, :])
                    nc.vector.tensor_mul(num[:, :], se[:, :], t2[:, :])
                    # qq = v/alpha
                    nc.vector.tensor_mul(qq[:, :], num[:, :], den[:, :])

                    if k == NITER - 1:
                        nc.sync.dma_start(
                            out=out_d[:, :].rearrange("(h p) f -> p h f",
                                                      p=128),
                            in_=qq[:, :].rearrange("p (h f) -> p h f", f=CO))
                        continue

                    nc.scalar.activation(vbf[:, :], qq[:, :],
                                         mybir.ActivationFunctionType.Copy,
                                         scale=alpha / B)

                    # --- G matmul + P = Wg .* G (3 K-tiles per psum) ---
                    xn4 = XN[:, :].rearrange("p (h t q) -> p t h q",
                                             h=NBH, q=128)
                    for g in range(NG):
                        g_ps = gps.tile([128, GRP * CO], F32, tag="g")
                        for j in range(GRP):
                            t = GRP * g + j
                            for bh in range(NBH):
                                nc.tensor.matmul(
                                    g_ps[:, j * CO:(j + 1) * CO],
                                    xn4[:, t, bh, :],
                                    vbf[:, bh * CO:(bh + 1) * CO],
                                    start=(bh == 0), stop=(bh == NBH - 1))
                        nc.vector.tensor_tensor(
                            Psb[:, g * GRP * CO:(g + 1) * GRP * CO],
                            Wg[:, g * GRP * CO:(g + 1) * GRP * CO],
                            g_ps[:, :], op=mybir.AluOpType.mult)
                        # indicator matmul reduces i; strided reduce over o
                        cp_ps = aps.tile([RT, GRP * CO], F32, tag="cp")
                        nc.tensor.matmul(
                            cp_ps[:, :], CDm[:, :],
                            Psb[:, g * GRP * CO:(g + 1) * GRP * CO],
                            start=True, stop=True)
                        nc.vector.tensor_reduce(
                            abar[:, g * GRP * C:(g + 1) * GRP * C]
                            .rearrange("m (t c) -> m t c", c=C),
                            cp_ps[:, :].rearrange("m (t o c) -> m t c o",
                                                  o=O, c=C),
                            axis=mybir.AxisListType.X,
                            op=mybir.AluOpType.add)

                    # --- b update (abar is complete locally) ---
                    if k == 0:
                        nc.scalar.copy(b_sb[:, :], abar[:, :])
                    else:
                        nc.vector.tensor_add(b_sb[:, :], b_sb[:, :],
                                             abar[:, :])

    nc.compile()
    return nc


def _host_inputs(x, W):
    rep = (np.arange(128)[None, :] // I ==
           np.arange(RT)[:, None]).astype(np.float32)
    cd = (np.arange(128)[:, None] // I ==
          np.arange(RT)[None, :]).astype(ml_dtypes.bfloat16)
    in_maps = []
    for cidx in range(NCORES):
        xs = x[:, cidx * RL:(cidx + 1) * RL, :].reshape(B, KL)
        Ws = W[cidx * RL:(cidx + 1) * RL]
        wg = np.ascontiguousarray(
            Ws.transpose(0, 3, 2, 1).reshape(KL, CO)).astype(
            ml_dtypes.bfloat16)
        xt = np.ascontiguousarray(
            xs.T.reshape(KT, 128, NBH, 128).transpose(1, 0, 2, 3)
            .reshape(128, KT * B)).astype(ml_dtypes.bfloat16)
        xn = np.ascontiguousarray(
            xs.reshape(NBH, 128, KL).transpose(1, 0, 2)
            .reshape(128, NBH * KL)).astype(ml_dtypes.bfloat16)
        in_maps.append({"wg": wg, "xt": xt, "xn": xn,
                        "rep": rep, "cd": cd})
    return in_maps


def kernel(x, W):
    x = np.ascontiguousarray(np.asarray(x, dtype=np.float32))
    W = np.ascontiguousarray(np.asarray(W, dtype=np.float32))
    assert x.shape == (B, R, I) and W.shape == (R, C, O, I)
    if "nc" not in _CACHE:
        _CACHE["nc"] = _build()
    nc = _CACHE["nc"]
    in_maps = _host_inputs(x, W)
    res = bass_utils.run_bass_kernel_spmd(nc, in_maps,
                                          core_ids=list(range(NCORES)))
    # every core holds the identical full output; take core 0
    v = res.results[0]["out"].reshape(B, O, C).transpose(0, 2, 1)
    return np.ascontiguousarray(v)[..., None].astype(np.float32)


# revision 9
# speedup vs baseline: 4.1108x; 4.1108x over previous
"""DigitCaps (CapsNet dynamic routing) kernel for 8 Trainium2 NeuronCores.

Reference math:
  u_hat[b,r,c,o] = sum_i W[r,c,o,i] * x[b,r,i]
  b_ij = 0;  3 routing iterations:
     c = softmax_r(b);  s[b,c,o] = sum_r c[r,c] u_hat[b,r,c,o];
     v = squash(s);     b += mean_b(sum_o u_hat[b,r,c,o] v[b,c,o])
  returns v[..., None]  (256, 10, 16, 1)

Strategy: ROUTE-parallel over r (144 routes per core), batch kept whole
(B=256) on every core; u_hat never materialized.  Local contraction dim
K=(r_local,i)=1152 (9 tiles of 128):
    s-matmul:  s_un[b,(o,c)] = sum_K XT[K,b] * (exp(b_ij)-scaled Wg)
               + AllReduce(s_un) over the 8 cores
    softmax denominators are DEFERRED: s_un uses unnormalized exp(b)
               weights; the per-class Z=sum_r exp(b) (tiny (1,C)
               AllReduce, computed on gpsimd/DMA queues concurrently
               with the s-matmuls) enters via the squash identity
                  v = s|s| / (s^2 + Z^2)
               which is exact for s = Z*s_true (and Z=R at iter 0).
    G-matmul:  G[K,(o,c)] = sum_b XN[b,K] * v[b,(o,c)]
    agreement: P = Wg .* G; an indicator matmul (CD, preloaded with
               1/B to apply the batch mean for free) reduces i inside
               each 128-partition tile; a strided tensor_reduce over o
               gives abar (complete locally -- no collective).
Engine budget is what matters on this backend (per-instruction cost
dominates; engines run concurrently): the tensor queue carries only the
unavoidable matmuls (18 s + 18 G + 3 COMP + 1 crep per iteration);
partition sum/broadcast for the softmax run on gpsimd; all DMAs are
fully contiguous (host prepares every layout).
"""
import sys
if '/opt/trn_rl_repo' not in sys.path:
    sys.path.insert(0, '/opt/trn_rl_repo')
import numpy as np
import ml_dtypes

import concourse.bass as bass
import concourse.bacc as bacc
import concourse.mybir as mybir
import concourse.bass_isa as bass_isa
import concourse.tile as tile
from concourse import bass_utils

BF16 = mybir.dt.bfloat16
F32 = mybir.dt.float32

B, R, C, O, I = 256, 1152, 10, 16, 8
NCORES = 8
RL = R // NCORES          # 144 local routes
KL = RL * I               # 1152 local contraction
KT = KL // 128            # 9 K tiles
RT = 128 // I             # 16 routes per K tile
CO = C * O                # 160, free order (o,c): idx = o*C + c
NBH = B // 128            # 2 batch halves
NITER = 3
GRP = 3                   # K tiles per PSUM group (G/COMP phase)
NG = KT // GRP            # 3 groups

_CACHE = {}


def _build(n_cores=NCORES, reps=1):
    nc = bacc.Bacc("TRN2", target_bir_lowering=False, debug=False,
                   num_devices=n_cores)
    wg_d = nc.dram_tensor("wg", [128, KT * CO], BF16, kind="ExternalInput")
    xt_d = nc.dram_tensor("xt", [128, KT * B], BF16, kind="ExternalInput")
    xn_d = nc.dram_tensor("xn", [128, NBH * KL], BF16, kind="ExternalInput")
    rep_d = nc.dram_tensor("rep", [RT, 128], F32, kind="ExternalInput")
    cd_d = nc.dram_tensor("cd", [128, RT], BF16, kind="ExternalInput")
    out_d = nc.dram_tensor("out", [128, NBH * CO], F32,
                           kind="ExternalOutput")

    with tile.TileContext(nc) as tc:
        with (
            tc.tile_pool(name="big", bufs=1) as big,
            tc.tile_pool(name="small", bufs=1) as small,
            tc.tile_pool(name="sps", bufs=1, space="PSUM") as sps,
            tc.tile_pool(name="gps", bufs=2, space="PSUM") as gps,
            tc.tile_pool(name="aps", bufs=1, space="PSUM") as aps,
            tc.tile_pool(name="zps", bufs=1, space="PSUM") as zps_pool,
            tc.tile_pool(name="dram", bufs=4, space="DRAM") as dram,
        ):
            Wg = big.tile([128, KT * CO], BF16, tag="Wg")
            Wp = big.tile([128, KT * CO], BF16, tag="Wp")
            Psb = big.tile([128, KT * CO], BF16, tag="Psb")
            XT = big.tile([128, KT * B], BF16, tag="XT")
            XN = big.tile([128, NBH * KL], BF16, tag="XN")
            REP = big.tile([RT, 128], F32, tag="REP")
            CDm = big.tile([128, RT], BF16, tag="CDm")

            se = big.tile([128, NBH * CO], F32, tag="se")
            t1 = big.tile([128, NBH * CO], F32, tag="t1")
            t2 = big.tile([128, NBH * CO], F32, tag="t2")
            den = big.tile([128, NBH * CO], F32, tag="den")
            num = big.tile([128, NBH * CO], F32, tag="num")
            qq = big.tile([128, NBH * CO], F32, tag="qq")
            vbf = big.tile([128, NBH * CO], BF16, tag="vbf")
            Z2bc = big.tile([128, C], F32, tag="Z2bc")

            b_sb = small.tile([RT, KT * C], F32, tag="b")
            expb = small.tile([RT, KT * C], F32, tag="expb")
            zpr = small.tile([RT, KT * C], F32, tag="zpr")
            abar = small.tile([RT, KT * C], F32, tag="abar")
            zr = small.tile([1, C], F32, tag="zr")
            zr2 = small.tile([1, C], F32, tag="zr2")
            zsq = small.tile([1, C], F32, tag="zsq")

            for rep in range(reps):
                nc.sync.dma_start(out=Wg[:, :], in_=wg_d[:, :])
                nc.sync.dma_start(out=XT[:, :], in_=xt_d[:, :])
                nc.sync.dma_start(out=XN[:, :], in_=xn_d[:, :])
                nc.sync.dma_start(out=REP[:, :], in_=rep_d[:, :])
                nc.sync.dma_start(out=CDm[:, :], in_=cd_d[:, :])

                xt4 = XT[:, :].rearrange("p (t h b) -> p t h b",
                                         h=NBH, b=128)
                xn4 = XN[:, :].rearrange("p (h t q) -> p t h q",
                                         h=NBH, q=128)

                for k in range(NITER):
                    if k > 0:
                        # exp(b); fold into weights (Z deferred to squash)
                        nc.scalar.activation(expb[:, :], b_sb[:, :],
                                             mybir.ActivationFunctionType.Exp)
                        crep = zps_pool.tile([128, KT * C], F32, tag="crep")
                        nc.tensor.matmul(crep[:, :], REP[:, :], expb[:, :],
                                         start=True, stop=True)
                        nc.vector.tensor_tensor(
                            Wp[:, :].rearrange("p (t o c) -> p t o c",
                                               o=O, c=C),
                            Wg[:, :].rearrange("p (t o c) -> p t o c",
                                               o=O, c=C),
                            crep[:, :].rearrange("p (t c) -> p t c", c=C)
                            .unsqueeze(2).broadcast_to((128, KT, O, C)),
                            op=mybir.AluOpType.mult)

                    # s matmul (tensor queue: the long block)
                    mov = Wg if k == 0 else Wp
                    s_ps = sps.tile([128, NBH * CO], F32, tag="s")
                    for bh in range(NBH):
                        for t in range(KT):
                            nc.tensor.matmul(
                                s_ps[:, bh * CO:(bh + 1) * CO],
                                xt4[:, t, bh, :],
                                mov[:, t * CO:(t + 1) * CO],
                                start=(t == 0), stop=(t == KT - 1))

                    if k > 0:
                        # Z = sum_r exp(b) via gpsimd+DMA+collective,
                        # concurrent with the s matmuls above
                        nc.gpsimd.partition_all_reduce(
                            zpr[:, :], expb[:, :], channels=RT,
                            reduce_op=bass_isa.ReduceOp.add)
                        nc.vector.tensor_reduce(
                            zr[:, :],
                            zpr[0:1, :].rearrange("q (t c) -> q c t", c=C),
                            axis=mybir.AxisListType.X,
                            op=mybir.AluOpType.add)
                        z_in = dram.tile([1, C], F32, tag="zin")
                        z_out = dram.tile([1, C], F32, tag="zout")
                        nc.sync.dma_start(out=z_in[:, :], in_=zr[:, :])
                        nc.gpsimd.collective_compute(
                            "AllReduce", mybir.AluOpType.add,
                            replica_groups=[list(range(n_cores))],
                            ins=[z_in[:, :].opt()],
                            outs=[z_out[:, :].opt()])
                        nc.sync.dma_start(out=zr2[:, :], in_=z_out[:, :])
                        nc.vector.tensor_mul(zsq[:, :], zr2[:, :],
                                             zr2[:, :])
                        nc.gpsimd.partition_broadcast(Z2bc[:, :],
                                                      zsq[:, :],
                                                      channels=128)

                    # AllReduce s over all cores (sum over all routes)
                    nc.scalar.copy(se[:, :], s_ps[:, :])
                    s_in = dram.tile([128, NBH * CO], F32, tag="sin")
                    s_out = dram.tile([128, NBH * CO], F32, tag="sout")
                    nc.sync.dma_start(out=s_in[:, :], in_=se[:, :])
                    nc.gpsimd.collective_compute(
                        "AllReduce", mybir.AluOpType.add,
                        replica_groups=[list(range(n_cores))],
                        ins=[s_in[:, :].opt()],
                        outs=[s_out[:, :].opt()])
                    nc.sync.dma_start(out=se[:, :], in_=s_out[:, :])

                    # squash: v = s|s| / (s^2 + Z^2)  (exact; Z=R at k=0)
                    nc.vector.tensor_mul(t1[:, :], se[:, :], se[:, :])
                    if k == 0:
                        nc.vector.tensor_scalar_add(den[:, :], t1[:, :],
                                                    float(R) * float(R))
                    else:
                        nc.vector.tensor_tensor(
                            den[:, :].rearrange("p (h o c) -> p h o c",
                                                h=NBH, o=O, c=C),
                            t1[:, :].rearrange("p (h o c) -> p h o c",
                                               h=NBH, o=O, c=C),
                            Z2bc[:, :].unsqueeze(1).unsqueeze(1)
                            .broadcast_to((128, NBH, O, C)),
                            op=mybir.AluOpType.add)
                    nc.vector.reciprocal(den[:, :], den[:, :])
                    nc.scalar.activation(t2[:, :], se[:, :],
                                         mybir.ActivationFunctionType.Abs)
                    nc.vector.tensor_mul(num[:, :], se[:, :], t2[:, :])
                    nc.vector.tensor_mul(qq[:, :], num[:, :], den[:, :])

                    if k == NITER - 1:
                        nc.sync.dma_start(out=out_d[:, :], in_=qq[:, :])
                        continue

                    nc.scalar.copy(vbf[:, :], qq[:, :])

                    # G matmul + P = Wg .* G + agreement reduce
                    tgt = b_sb if k == 0 else abar
                    for g in range(NG):
                        g_ps = gps.tile([128, GRP * CO], F32, tag="g")
                        for j in range(GRP):
                            t = GRP * g + j
                            for bh in range(NBH):
                                nc.tensor.matmul(
                                    g_ps[:, j * CO:(j + 1) * CO],
                                    xn4[:, t, bh, :],
                                    vbf[:, bh * CO:(bh + 1) * CO],
                                    start=(bh == 0), stop=(bh == NBH - 1))
                        nc.vector.tensor_tensor(
                            Psb[:, g * GRP * CO:(g + 1) * GRP * CO],
                            Wg[:, g * GRP * CO:(g + 1) * GRP * CO],
                            g_ps[:, :], op=mybir.AluOpType.mult)
                        # CD carries 1/B: abar = mean_b(a) directly
                        cp_ps = aps.tile([RT, GRP * CO], F32, tag="cp")
                        nc.tensor.matmul(
                            cp_ps[:, :], CDm[:, :],
                            Psb[:, g * GRP * CO:(g + 1) * GRP * CO],
                            start=True, stop=True)
                        nc.vector.tensor_reduce(
                            tgt[:, g * GRP * C:(g + 1) * GRP * C]
                            .rearrange("m (t c) -> m t c", c=C),
                            cp_ps[:, :].rearrange("m (t o c) -> m t c o",
                                                  o=O, c=C),
                            axis=mybir.AxisListType.X,
                            op=mybir.AluOpType.add)

                    if k > 0:
                        nc.vector.tensor_add(b_sb[:, :], b_sb[:, :],
                                             abar[:, :])

    nc.compile()
    return nc


def _host_inputs(x, W):
    rep = (np.arange(128)[None, :] // I ==
           np.arange(RT)[:, None]).astype(np.float32)
    cd = ((np.arange(128)[:, None] // I ==
           np.arange(RT)[None, :]).astype(np.float32) / B).astype(
        ml_dtypes.bfloat16)
    in_maps = []
    for cidx in range(NCORES):
        xs = x[:, cidx * RL:(cidx + 1) * RL, :].reshape(B, KL)
        Ws = W[cidx * RL:(cidx + 1) * RL]
        wg = np.ascontiguousarray(
            Ws.transpose(0, 3, 2, 1).reshape(KT, 128, CO)
            .transpose(1, 0, 2).reshape(128, KT * CO)).astype(
            ml_dtypes.bfloat16)
        xt = np.ascontiguousarray(
            xs.T.reshape(KT, 128, NBH, 128).transpose(1, 0, 2, 3)
            .reshape(128, KT * B)).astype(ml_dtypes.bfloat16)
        xn = np.ascontiguousarray(
            xs.reshape(NBH, 128, KL).transpose(1, 0, 2)
            .reshape(128, NBH * KL)).astype(ml_dtypes.bfloat16)
        in_maps.append({"wg": wg, "xt": xt, "xn": xn,
                        "rep": rep, "cd": cd})
    return in_maps


def kernel(x, W):
    x = np.ascontiguousarray(np.asarray(x, dtype=np.float32))
    W = np.ascontiguousarray(np.asarray(W, dtype=np.float32))
    assert x.shape == (B, R, I) and W.shape == (R, C, O, I)
    if "nc" not in _CACHE:
        _CACHE["nc"] = _build()
    nc = _CACHE["nc"]
    in_maps = _host_inputs(x, W)
    res = bass_utils.run_bass_kernel_spmd(nc, in_maps,
                                          core_ids=list(range(NCORES)))
    # every core holds the identical full output; take core 0.
    # out[p, (h, o*C+c)] = v[h*128+p, o*C+c]
    v = res.results[0]["out"].reshape(128, NBH, O, C)
    v = v.transpose(1, 0, 3, 2).reshape(B, C, O)
    return np.ascontiguousarray(v)[..., None].astype(np.float32)


# revision 10
# speedup vs baseline: 4.5110x; 1.0973x over previous
"""DigitCaps (CapsNet dynamic routing) kernel for 8 Trainium2 NeuronCores.

Reference math:
  u_hat[b,r,c,o] = sum_i W[r,c,o,i] * x[b,r,i]
  b_ij = 0;  3 routing iterations:
     c = softmax_r(b);  s[b,c,o] = sum_r c[r,c] u_hat[b,r,c,o];
     v = squash(s);     b += mean_b(sum_o u_hat[b,r,c,o] v[b,c,o])
  returns v[..., None]  (256, 10, 16, 1)

Strategy: ROUTE-parallel over r (144 routes per core), full batch per
core; u_hat never materialized; the softmax numerator exp(b) is folded
into the weights and the denominator Z enters exactly via
     v = s|s| / (s^2 + Z^2)        (Z = R at iteration 0)
so the tiny (1,C) Z AllReduce runs concurrently with the s-matmuls.

Precision split (validated in work/sim_fp8.py):
  - the OUTPUT iteration's s-matmul runs in bf16 (max-rel 4.2e-3)
  - the two ROUTING-only s-matmuls and the agreement G-matmul run in
    fp8e4m3 with DoubleRow perf mode: each instruction contracts
    2x128 partitions, halving the PE instruction count.
  - agreement: P = Wg .* G (bf16); indicator matmul CD2 (128->128,
    entries 1/B) both reduces i and replicates the result over i, so
    b_ij lives i-replicated on 128 partitions and feeds the weight
    fold with no further replication matmul.

This backend's wall time is per-instruction; the PE queue dominates
(ldweights + matmult per matmul), so the design minimizes matmul count:
10+10+18 s-matmuls, 9+9 G, 3+3 COMP = 62 total.
"""
import sys
if '/opt/trn_rl_repo' not in sys.path:
    sys.path.insert(0, '/opt/trn_rl_repo')
import numpy as np
import ml_dtypes

import concourse.bass as bass
import concourse.bacc as bacc
import concourse.mybir as mybir
import concourse.bass_isa as bass_isa
import concourse.tile as tile
from concourse import bass_utils

BF16 = mybir.dt.bfloat16
F32 = mybir.dt.float32
FP8 = mybir.dt.float8e4
DR = mybir.MatmulPerfMode.DoubleRow

B, R, C, O, I = 256, 1152, 10, 16, 8
NCORES = 8
RL = R // NCORES          # 144 local routes
KL = RL * I               # 1152 local contraction
KT = KL // 128            # 9 K tiles
KT8 = 10                  # padded to 5 DoubleRow pairs
CO = C * O                # 160, free order (o,c): idx = o*C + c
NBH = B // 128            # 2 batch halves
NITER = 3
GRP = 3                   # K tiles per PSUM group (G/COMP phase)
NG = KT // GRP            # 3 groups

_CACHE = {}


def _build(n_cores=NCORES, reps=1):
    nc = bacc.Bacc("TRN2", target_bir_lowering=False, debug=False,
                   num_devices=n_cores)
    wg_d = nc.dram_tensor("wg", [128, KT * CO], BF16, kind="ExternalInput")
    wg8_d = nc.dram_tensor("wg8", [128, KT8 * CO], FP8, kind="ExternalInput")
    xt8_d = nc.dram_tensor("xt8", [128, KT8 * B], FP8, kind="ExternalInput")
    xtb_d = nc.dram_tensor("xtb", [128, KT * B], BF16, kind="ExternalInput")
    xn8_d = nc.dram_tensor("xn8", [128, KT * B], FP8, kind="ExternalInput")
    cd2_d = nc.dram_tensor("cd2", [128, 128], BF16, kind="ExternalInput")
    out_d = nc.dram_tensor("out", [128, NBH * CO], F32,
                           kind="ExternalOutput")

    with tile.TileContext(nc) as tc:
        with (
            tc.tile_pool(name="big", bufs=1) as big,
            tc.tile_pool(name="small", bufs=1) as small,
            tc.tile_pool(name="sps", bufs=1, space="PSUM") as sps,
            tc.tile_pool(name="gps", bufs=2, space="PSUM") as gps,
            tc.tile_pool(name="aps", bufs=1, space="PSUM") as aps,
            tc.tile_pool(name="dram", bufs=4, space="DRAM") as dram,
        ):
            Wg = big.tile([128, KT * CO], BF16, tag="Wg")
            Wg8 = big.tile([128, KT8 * CO], FP8, tag="Wg8")
            Wp8 = big.tile([128, KT8 * CO], FP8, tag="Wp8")
            Wpb = big.tile([128, KT * CO], BF16, tag="Wpb")
            Psb = big.tile([128, KT * CO], BF16, tag="Psb")
            XT8 = big.tile([128, KT8 * B], FP8, tag="XT8")
            XTb = big.tile([128, KT * B], BF16, tag="XTb")
            XN8 = big.tile([128, KT * B], FP8, tag="XN8")
            CD2 = big.tile([128, 128], BF16, tag="CD2")

            se = big.tile([128, NBH * CO], F32, tag="se")
            t1 = big.tile([128, NBH * CO], F32, tag="t1")
            t2 = big.tile([128, NBH * CO], F32, tag="t2")
            den = big.tile([128, NBH * CO], F32, tag="den")
            num = big.tile([128, NBH * CO], F32, tag="num")
            qq = big.tile([128, NBH * CO], F32, tag="qq")
            vbf8 = big.tile([128, NBH * CO], FP8, tag="vbf8")
            Z2bc = big.tile([128, C], F32, tag="Z2bc")

            b_sb = small.tile([128, KT * C], F32, tag="b")
            expb = small.tile([128, KT * C], F32, tag="expb")
            zpr = small.tile([128, KT * C], F32, tag="zpr")
            abar = small.tile([128, KT * C], F32, tag="abar")
            zr = small.tile([1, C], F32, tag="zr")
            zr2 = small.tile([1, C], F32, tag="zr2")
            zsq = small.tile([1, C], F32, tag="zsq")

            for rep in range(reps):
                nc.sync.dma_start(out=Wg[:, :], in_=wg_d[:, :])
                nc.sync.dma_start(out=Wg8[:, :], in_=wg8_d[:, :])
                nc.sync.dma_start(out=XT8[:, :], in_=xt8_d[:, :])
                nc.sync.dma_start(out=XTb[:, :], in_=xtb_d[:, :])
                nc.sync.dma_start(out=XN8[:, :], in_=xn8_d[:, :])
                nc.sync.dma_start(out=CD2[:, :], in_=cd2_d[:, :])
                # zero the padding K tile of the folded fp8 weights
                nc.vector.memset(Wp8[:, KT * CO:], 0.0)

                xt8v = XT8[:, :].rearrange("p (t h b) -> p t h b",
                                           h=NBH, b=128)
                xtbv = XTb[:, :].rearrange("p (t h b) -> p t h b",
                                           h=NBH, b=128)
                xn8v = XN8[:, :].rearrange("p (t h q) -> p t h q",
                                           h=NBH, q=128)
                wg8v = Wg8[:, :].rearrange("p (t f) -> p t f", f=CO)
                wp8v = Wp8[:, :].rearrange("p (t f) -> p t f", f=CO)

                for k in range(NITER):
                    last = k == NITER - 1
                    if k > 0:
                        # fold exp(b) into the weights (Z deferred)
                        nc.scalar.activation(expb[:, :], b_sb[:, :],
                                             mybir.ActivationFunctionType.Exp)
                        ex3 = expb[:, :].rearrange(
                            "p (t c) -> p t c", c=C).unsqueeze(2) \
                            .broadcast_to((128, KT, O, C))
                        wgv = Wg[:, :].rearrange("p (t o c) -> p t o c",
                                                 o=O, c=C)
                        if last:
                            nc.vector.tensor_tensor(
                                Wpb[:, :].rearrange(
                                    "p (t o c) -> p t o c", o=O, c=C),
                                wgv, ex3, op=mybir.AluOpType.mult)
                        else:
                            nc.vector.tensor_tensor(
                                Wp8[:, :KT * CO].rearrange(
                                    "p (t o c) -> p t o c", o=O, c=C),
                                wgv, ex3, op=mybir.AluOpType.mult)

                    # s matmul (the long PE block)
                    s_ps = sps.tile([128, NBH * CO], F32, tag="s")
                    if last:
                        for bh in range(NBH):
                            for t in range(KT):
                                nc.tensor.matmul(
                                    s_ps[:, bh * CO:(bh + 1) * CO],
                                    xtbv[:, t, bh, :],
                                    Wpb[:, t * CO:(t + 1) * CO],
                                    start=(t == 0), stop=(t == KT - 1))
                    else:
                        mov = wg8v if k == 0 else wp8v
                        for bh in range(NBH):
                            for gp in range(KT8 // 2):
                                nc.tensor.matmul(
                                    s_ps[:, bh * CO:(bh + 1) * CO],
                                    xt8v[:, 2 * gp:2 * gp + 2, bh, :],
                                    mov[:, 2 * gp:2 * gp + 2, :],
                                    start=(gp == 0),
                                    stop=(gp == KT8 // 2 - 1),
                                    perf_mode=DR)

                    if k > 0:
                        # Z = sum_r exp(b): gpsimd+DMA+collective path,
                        # concurrent with the s matmuls (b is stored
                        # i-replicated so the partition sum is 8Z)
                        nc.gpsimd.partition_all_reduce(
                            zpr[:, :], expb[:, :], channels=128,
                            reduce_op=bass_isa.ReduceOp.add)
                        nc.vector.tensor_reduce(
                            zr[:, :],
                            zpr[0:1, :].rearrange("q (t c) -> q c t", c=C),
                            axis=mybir.AxisListType.X,
                            op=mybir.AluOpType.add)
                        z_in = dram.tile([1, C], F32, tag="zin")
                        z_out = dram.tile([1, C], F32, tag="zout")
                        nc.sync.dma_start(out=z_in[:, :], in_=zr[:, :])
                        nc.gpsimd.collective_compute(
                            "AllReduce", mybir.AluOpType.add,
                            replica_groups=[list(range(n_cores))],
                            ins=[z_in[:, :].opt()],
                            outs=[z_out[:, :].opt()])
                        nc.sync.dma_start(out=zr2[:, :], in_=z_out[:, :])
                        # zr2 = 8*Z  ->  zsq = Z^2
                        nc.scalar.activation(
                            zsq[:, :], zr2[:, :],
                            mybir.ActivationFunctionType.Square,
                            scale=1.0 / 8.0)
                        nc.gpsimd.partition_broadcast(Z2bc[:, :],
                                                      zsq[:, :],
                                                      channels=128)

                    # AllReduce s over all cores (sum over all routes)
                    nc.scalar.copy(se[:, :], s_ps[:, :])
                    s_in = dram.tile([128, NBH * CO], F32, tag="sin")
                    s_out = dram.tile([128, NBH * CO], F32, tag="sout")
                    nc.sync.dma_start(out=s_in[:, :], in_=se[:, :])
                    nc.gpsimd.collective_compute(
                        "AllReduce", mybir.AluOpType.add,
                        replica_groups=[list(range(n_cores))],
                        ins=[s_in[:, :].opt()],
                        outs=[s_out[:, :].opt()])
                    nc.sync.dma_start(out=se[:, :], in_=s_out[:, :])

                    # squash: v = s|s| / (s^2 + Z^2)  (exact; Z=R at k=0)
                    nc.vector.tensor_mul(t1[:, :], se[:, :], se[:, :])
                    if k == 0:
                        nc.vector.tensor_scalar_add(den[:, :], t1[:, :],
                                                    float(R) * float(R))
                    else:
                        nc.vector.tensor_tensor(
                            den[:, :].rearrange("p (h o c) -> p h o c",
                                                h=NBH, o=O, c=C),
                            t1[:, :].rearrange("p (h o c) -> p h o c",
                                               h=NBH, o=O, c=C),
                            Z2bc[:, :].unsqueeze(1).unsqueeze(1)
                            .broadcast_to((128, NBH, O, C)),
                            op=mybir.AluOpType.add)
                    nc.vector.reciprocal(den[:, :], den[:, :])
                    nc.scalar.activation(t2[:, :], se[:, :],
                                         mybir.ActivationFunctionType.Abs)
                    nc.vector.tensor_mul(num[:, :], se[:, :], t2[:, :])
                    nc.vector.tensor_mul(qq[:, :], num[:, :], den[:, :])

                    if last:
                        nc.sync.dma_start(out=out_d[:, :], in_=qq[:, :])
                        continue

                    nc.scalar.copy(vbf8[:, :], qq[:, :])

                    # G matmul (fp8 DoubleRow: both batch halves per
                    # instruction) + P = Wg .* G + agreement reduce
                    vb3 = vbf8[:, :].rearrange("p (h f) -> p h f", f=CO)
                    tgt = b_sb if k == 0 else abar
                    for g in range(NG):
                        g_ps = gps.tile([128, GRP * CO], F32, tag="g")
                        for j in range(GRP):
                            t = GRP * g + j
                            nc.tensor.matmul(
                                g_ps[:, j * CO:(j + 1) * CO],
                                xn8v[:, t, :, :],
                                vb3[:, :, :],
                                start=True, stop=True,
                                perf_mode=DR)
                        nc.vector.tensor_tensor(
                            Psb[:, g * GRP * CO:(g + 1) * GRP * CO],
                            Wg[:, g * GRP * CO:(g + 1) * GRP * CO],
                            g_ps[:, :], op=mybir.AluOpType.mult)
                        # CD2 carries 1/B and replicates over i
                        cp_ps = aps.tile([128, GRP * CO], F32, tag="cp")
                        nc.tensor.matmul(
                            cp_ps[:, :], CD2[:, :],
                            Psb[:, g * GRP * CO:(g + 1) * GRP * CO],
                            start=True, stop=True)
                        nc.vector.tensor_reduce(
                            tgt[:, g * GRP * C:(g + 1) * GRP * C]
                            .rearrange("m (t c) -> m t c", c=C),
                            cp_ps[:, :].rearrange("m (t o c) -> m t c o",
                                                  o=O, c=C),
                            axis=mybir.AxisListType.X,
                            op=mybir.AluOpType.add)

                    if k > 0:
                        nc.vector.tensor_add(b_sb[:, :], b_sb[:, :],
                                             abar[:, :])

    nc.compile()
    return nc


def _host_inputs(x, W):
    F8 = ml_dtypes.float8_e4m3
    cd2 = ((np.arange(128)[:, None] // I ==
            np.arange(128)[None, :] // I).astype(np.float32) / B).astype(
        ml_dtypes.bfloat16)
    in_maps = []
    for cidx in range(NCORES):
        xs = x[:, cidx * RL:(cidx + 1) * RL, :].reshape(B, KL)
        Ws = W[cidx * RL:(cidx + 1) * RL]
        wgk = Ws.transpose(0, 3, 2, 1).reshape(KT, 128, CO)
        wg = np.ascontiguousarray(
            wgk.transpose(1, 0, 2).reshape(128, KT * CO)).astype(
            ml_dtypes.bfloat16)
        wg8 = np.zeros((128, KT8, CO), np.float32)
        wg8[:, :KT, :] = wgk.transpose(1, 0, 2)
        wg8 = np.ascontiguousarray(wg8.reshape(128, KT8 * CO)).astype(F8)
        xtk = xs.T.reshape(KT, 128, NBH, 128).transpose(1, 0, 2, 3)
        xtb = np.ascontiguousarray(
            xtk.reshape(128, KT * B)).astype(ml_dtypes.bfloat16)
        xt8 = np.zeros((128, KT8, NBH, 128), np.float32)
        xt8[:, :KT] = xtk
        xt8 = np.ascontiguousarray(xt8.reshape(128, KT8 * B)).astype(F8)
        # xn8[p, t, h, q] = xs[h*128+p, t*128+q]
        xn8 = np.ascontiguousarray(
            xs.reshape(NBH, 128, KT, 128).transpose(1, 2, 0, 3)
            .reshape(128, KT * B)).astype(F8)
        in_maps.append({"wg": wg, "wg8": wg8, "xt8": xt8, "xtb": xtb,
                        "xn8": xn8, "cd2": cd2})
    return in_maps


def kernel(x, W):
    x = np.ascontiguousarray(np.asarray(x, dtype=np.float32))
    W = np.ascontiguousarray(np.asarray(W, dtype=np.float32))
    assert x.shape == (B, R, I) and W.shape == (R, C, O, I)
    if "nc" not in _CACHE:
        _CACHE["nc"] = _build()
    nc = _CACHE["nc"]
    in_maps = _host_inputs(x, W)
    res = bass_utils.run_bass_kernel_spmd(nc, in_maps,
                                          core_ids=list(range(NCORES)))
    # every core holds the identical full output; take core 0.
    v = res.results[0]["out"].reshape(128, NBH, O, C)
    v = v.transpose(1, 0, 3, 2).reshape(B, C, O)
    return np.ascontiguousarray(v)[..., None].astype(np.float32)
